# revision 28
# baseline (speedup 1.0000x reference)
"""Trainium2 Bass kernel for nn_CRSDBlock (2-layer leaky-reservoir RNN).

Problem: B=32, T=1024, D=R=1024, 2 layers.
  per layer, per step t:
    r_new = (1-a)*r + a*tanh(x_t @ Wxr + h @ Whr)
    h_new = tanh(x_t @ Wxh + h @ Whh + r_new @ Wrh)

Strategy (8 NeuronCores):
  - Data-parallel over batch: each core gets b=4 batch rows, weights replicated.
    No collectives needed.
  - Per core / per layer:
      Phase B: precompute XR = x @ Wxr and XH = x @ Wxh for ALL timesteps as
               big efficient matmuls (they don't depend on the recurrence).
      Phase C: sequential loop over t with only the 3 recurrent matvecs
               (h@Whr, h@Whh, u@Wrh); every operand kept transposed
               (feature dim on partitions, batch on the free axis).
               Instead of r we track s = r @ Wrh: s' = (1-a)*s + a*(u @ Wrh).
      TensorE compute in bf16 (fp32 matmul is 4x slower), fp32 PSUM
      accumulation, fp32 state + pre-activations.
  - Final pass PE-transposes layer-2 output back to natural [b, t, d].

DRAM scratch layouts are partition-major [128, chunk, ...] so that each
For_i block's XR/XH loads and h stores are single 3D-AP DMAs (dynamic-offset
DMAs need per-engine registers; keeping them few avoids register exhaustion).
Feature index convention everywhere: feat = chunk * 128 + partition.
"""

import numpy as np

import concourse.bass as bass
import concourse.bacc as bacc_mod
import concourse.mybir as mybir
from concourse.tile import TileContext
from concourse.bass import ds
from concourse.bass_utils import run_bass_kernel_spmd
from concourse.masks import make_identity

FP32 = mybir.dt.float32
BF16 = mybir.dt.bfloat16
AF = mybir.ActivationFunctionType
ALU = mybir.AluOpType

P = 128          # partitions
B_LOC = 4        # batch per core (32 / 8)
D = 1024         # feature dim
NCH = D // P     # 8 chunks of 128
ALPHA = 0.1
N_CORES = 8



class _WView:
    def __init__(self, w_all, layer):
        self.w_all = w_all
        self.layer = layer

    def __getitem__(self, nm):
        return self.w_all[(self.layer, nm)]


def build_nc(T=1024, steps_per_body=64, layers=2):
    assert T % steps_per_body == 0 and steps_per_body % 2 == 0
    assert T % 512 == 0
    SUB = steps_per_body // 2          # half-body granularity for XR/XH blocks
    assert 512 % SUB == 0
    nc = bacc_mod.Bacc(None)

    x = nc.declare_dram_parameter("x_seq", [B_LOC, T, D], FP32, isOutput=False)
    Wxh = nc.declare_dram_parameter("W_xh", [2, D, D], FP32, isOutput=False)
    Whh = nc.declare_dram_parameter("W_hh", [2, D, D], FP32, isOutput=False)
    Wrh = nc.declare_dram_parameter("W_rh", [2, D, D], FP32, isOutput=False)
    Wxr = nc.declare_dram_parameter("W_xr", [2, D, D], FP32, isOutput=False)
    Whr = nc.declare_dram_parameter("W_hr", [2, D, D], FP32, isOutput=False)
    out = nc.declare_dram_parameter("out", [B_LOC, T, D], FP32, isOutput=True)

    with TileContext(nc) as tc:
        with tc.tile_pool(name="dram", bufs=1, space="DRAM") as dram_pool, \
             tc.tile_pool(name="const", bufs=1) as const_pool:

            # persistent DRAM scratch (pool-managed so Tile tracks deps).
            # XR/XH/h outputs are stored pre-blocked by half-body (SUB steps)
            # so each recurrence block loads/stores with single fully
            # contiguous DMAs.
            NB = T // SUB
            xT = dram_pool.tile([P, NCH, B_LOC, T], BF16)   # x^T  (p, di_ch, b, t)
            h1T = dram_pool.tile([P, NB, NCH, SUB, B_LOC], BF16)  # L1 out, blocked (t,b)
            h2T = dram_pool.tile([P, NB, NCH, B_LOC, SUB], FP32)  # L2 out, blocked (b,t)
            xr1 = dram_pool.tile([P, NB, NCH, B_LOC, SUB], FP32)
            xh1 = dram_pool.tile([P, NB, NCH, B_LOC, SUB], FP32)
            xr2 = dram_pool.tile([P, NB, NCH, SUB, B_LOC], FP32)
            xh2 = dram_pool.tile([P, NB, NCH, SUB, B_LOC], FP32)

            ident = const_pool.tile([P, P], FP32)
            make_identity(nc, ident)
            ident_bf = const_pool.tile([P, P], BF16)
            nc.vector.tensor_copy(ident_bf, ident)

            # ---- Phase A: convert x to bf16 and transpose via PE into xT
            with tc.tile_pool(name="xconv", bufs=3) as xc_pool, \
                 tc.tile_pool(name="xconvp", bufs=4, space="PSUM") as xcp_pool:
                for b in range(B_LOC):
                    for tch in range(T // P):
                        tf = xc_pool.tile([P, D], FP32, tag="xc_f")
                        nc.sync.dma_start(out=tf, in_=x[b, tch * P:(tch + 1) * P, :])
                        tb = xc_pool.tile([P, D], BF16, tag="xc_b")
                        nc.vector.tensor_copy(tb, tf)
                        for di in range(NCH):
                            ptt = xcp_pool.tile([P, P], BF16, tag="xc_ps")
                            nc.tensor.transpose(ptt, tb[:, di * P:(di + 1) * P], ident_bf)
                            xo = xc_pool.tile([P, P], BF16, tag="xc_o")
                            nc.vector.tensor_copy(xo, ptt)
                            nc.sync.dma_start(
                                out=xT[:, di, b, tch * P:(tch + 1) * P], in_=xo)

            # ---- load + bf16-convert all weight matrices for both layers
            # up front, so layer-2 weight DMAs overlap layer-1 compute
            w_all = {}
            with tc.tile_pool(name="wpool", bufs=1) as wpool, \
                 tc.tile_pool(name="wtmp", bufs=3) as wtmp_pool:
                def load_w(layer, names):
                    for nm, srcw in names:
                        wt = wpool.tile([P, NCH, D], BF16, tag=f"w{layer}_{nm}")
                        w_all[(layer, nm)] = wt
                        for ki in range(NCH):
                            tf = wtmp_pool.tile([P, D], FP32, tag="wtmp")
                            nc.sync.dma_start(
                                out=tf, in_=srcw[layer, ki * P:(ki + 1) * P, :])
                            if nm == "rh":
                                # fold the leak coefficient into W_rh so the
                                # s-update is a single add
                                nc.vector.tensor_scalar_mul(wt[:, ki, :], tf, ALPHA)
                            else:
                                nc.vector.tensor_copy(wt[:, ki, :], tf)

                for layer in range(layers):
                    # xr/xh first (phase B needs only these; the bulky
                    # hr/hh/rh DMAs would otherwise queue ahead of phase B's
                    # rhs loads on the DMA engines)
                    load_w(layer, (("xr", Wxr), ("xh", Wxh)))
                    w_sb = _WView(w_all, layer)

                    # ---- Phase B: XR/XH precompute over all tokens
                    xr_d = xr1 if layer == 0 else xr2
                    xh_d = xh1 if layer == 0 else xh2
                    with tc.tile_pool(name=f"pb{layer}", bufs=3) as pb_pool, \
                         tc.tile_pool(name=f"pbp{layer}", bufs=4, space="PSUM") as pbp_pool:
                        if layer == 0:
                            spans = [(b, tsp) for b in range(B_LOC)
                                     for tsp in range(T // 512)]
                        else:
                            spans = list(range(B_LOC * T // 512))
                        for sp in spans:
                            xt = []
                            for ki in range(NCH):
                                tl = pb_pool.tile([P, 512], BF16, tag="pb_rhs",
                                                  bufs=2 * NCH)
                                if layer == 0:
                                    b, tsp = sp
                                    nc.sync.dma_start(
                                        out=tl,
                                        in_=xT[:, ki, b, tsp * 512:(tsp + 1) * 512])
                                else:
                                    # 512 consecutive tokens = 128 t x 4 b
                                    # = 128 // SUB consecutive sub-blocks
                                    nsb = P // SUB
                                    nc.sync.dma_start(
                                        out=tl,
                                        in_=h1T[:, sp * nsb:(sp + 1) * nsb, ki, :, :])
                                xt.append(tl)
                            for nm, dst in (("xr", xr_d), ("xh", xh_d)):
                                for mo in range(NCH):
                                    ps = pbp_pool.tile([P, 512], FP32, tag="pb_ps")
                                    for ki in range(NCH):
                                        nc.tensor.matmul(
                                            ps, w_sb[nm][:, ki, mo * P:(mo + 1) * P],
                                            xt[ki], start=(ki == 0),
                                            stop=(ki == NCH - 1))
                                    so = pb_pool.tile([P, 512], FP32, tag="pb_out")
                                    nc.vector.tensor_copy(so, ps)
                                    nsb = 512 // SUB if layer == 0 else P // SUB
                                    if layer == 0:
                                        b, tsp = sp
                                        nc.sync.dma_start(
                                            out=dst[:, tsp * nsb:(tsp + 1) * nsb,
                                                    mo, b, :],
                                            in_=so)
                                    else:
                                        nc.sync.dma_start(
                                            out=dst[:, sp * nsb:(sp + 1) * nsb,
                                                    mo, :, :],
                                            in_=so)

                    # recurrence weights: loaded after phase B is emitted so
                    # their DMAs overlap phase B / the previous layer's compute
                    load_w(layer, (("hr", Whr), ("hh", Whh), ("rh", Wrh)))
                    # ---- Phase C: the sequential recurrence
                    SPB = steps_per_body
                    with tc.tile_pool(name=f"st{layer}", bufs=1) as st_pool, \
                         tc.tile_pool(name=f"cb{layer}", bufs=1) as cb_pool, \
                         tc.tile_pool(name=f"cp{layer}", bufs=2, space="PSUM") as cp_pool, \
                         tc.tile_pool(name=f"cq{layer}", bufs=2) as cp2_pool:
                        hT_a = st_pool.tile([P, NCH, B_LOC], BF16, tag="hT_a")
                        hT_b = st_pool.tile([P, NCH, B_LOC], BF16, tag="hT_b")
                        uT = st_pool.tile([P, NCH, B_LOC], BF16, tag="uT")
                        s_sb = st_pool.tile([P, NCH, B_LOC], FP32, tag="s")
                        s09 = st_pool.tile([P, NCH, B_LOC], FP32, tag="s09")
                        nc.vector.memset(hT_a, 0.0)
                        nc.vector.memset(s_sb, 0.0)

                        NBODY = T // SPB
                        with tc.For_i(0, 2 * NBODY, 2, hint_engines=(mybir.EngineType.PE,)) as bv:
                            # bv = sub-block index; each body covers sub-blocks
                            # bv and bv+1.  Each load is one fully-contiguous
                            # DMA; the second half's loads overlap the first
                            # half's compute (separate tiles -> clean deps).
                            xr_s = xr1 if layer == 0 else xr2
                            xh_s = xh1 if layer == 0 else xh2
                            if layer == 0:
                                half_shape = [P, NCH, B_LOC, SUB]
                                hblk = cb_pool.tile([P, 2, NCH, SUB, B_LOC], FP32, tag="hblk")
                            else:
                                half_shape = [P, NCH, SUB, B_LOC]
                                hblk = cb_pool.tile([P, 2, NCH, B_LOC, SUB], FP32, tag="hblk")
                            xrb0 = cb_pool.tile(half_shape, FP32, tag="xrb0")
                            xrb1 = cb_pool.tile(half_shape, FP32, tag="xrb1")
                            xhb0 = cb_pool.tile(half_shape, FP32, tag="xhb0")
                            xhb1 = cb_pool.tile(half_shape, FP32, tag="xhb1")
                            nc.sync.dma_start(out=xrb0, in_=xr_s[:, ds(bv, 1)])
                            nc.sync.dma_start(out=xhb0, in_=xh_s[:, ds(bv, 1)])
                            nc.sync.dma_start(out=xrb1, in_=xr_s[:, ds(bv + 1, 1)])
                            nc.sync.dma_start(out=xhb1, in_=xh_s[:, ds(bv + 1, 1)])

                            for j in range(SPB):
                                h_in = hT_a if j % 2 == 0 else hT_b
                                h_out = hT_b if j % 2 == 0 else hT_a
                                xrb = xrb0 if j < SUB else xrb1
                                xhb = xhb0 if j < SUB else xhb1
                                jj = j % SUB
                                if layer == 0:
                                    xr_j = xrb[:, :, :, jj]
                                    xh_j = xhb[:, :, :, jj]
                                    hb_j = hblk[:, j // SUB, :, jj, :]
                                else:
                                    xr_j = xrb[:, :, jj, :]
                                    xh_j = xhb[:, :, jj, :]
                                    hb_j = hblk[:, j // SUB, :, :, jj]

                                # h @ Whr in mo-halves: each half's
                                # add+tanh starts after only that half's sem
                                # backlog, so u is ready well before the Wrh
                                # matmuls need it.  (Keep mo-outer groups —
                                # interleaved accumulation groups are broken.)
                                HCH = NCH // 2
                                psu_h = []
                                for half in range(2):
                                    pu = cp_pool.tile([P, HCH, B_LOC], FP32,
                                                      tag=f"psu{half}", bufs=1)
                                    psu_h.append(pu)
                                    for m in range(HCH):
                                        mo = half * HCH + m
                                        for ki in range(NCH):
                                            nc.tensor.matmul(
                                                pu[:, m, :],
                                                w_sb["hr"][:, ki, mo * P:(mo + 1) * P],
                                                h_in[:, ki, :], start=(ki == 0),
                                                stop=(ki == NCH - 1))
                                psh = cp_pool.tile([P, NCH, B_LOC], FP32, tag="psh", bufs=1)
                                for mo in range(NCH):
                                    for ki in range(NCH):
                                        nc.tensor.matmul(
                                            psh[:, mo, :],
                                            w_sb["hh"][:, ki, mo * P:(mo + 1) * P],
                                            h_in[:, ki, :], start=(ki == 0),
                                            stop=(ki == NCH - 1))
                                # s09 = 0.9 * s_prev  (off critical path; DVE
                                # runs it while the PE streams matmuls)
                                nc.vector.tensor_scalar_mul(s09, s_sb, 1.0 - ALPHA)
                                # u = tanh(psu + xr_j), per half
                                for half in range(2):
                                    sl = slice(half * HCH, (half + 1) * HCH)
                                    nc.vector.tensor_add(
                                        psu_h[half], psu_h[half], xr_j[:, sl, :])
                                    nc.scalar.activation(
                                        uT[:, sl, :], psu_h[half], AF.Tanh)
                                # q = psh + xh_j + 0.9*s  (q in SBUF so the
                                # tail adds have only one PSUM operand each)
                                q_sb = cp2_pool.tile([P, NCH, B_LOC], FP32, tag="q")
                                nc.vector.tensor_add(q_sb, psh, xh_j)
                                nc.vector.tensor_add(q_sb, q_sb, s09)
                                # pss = u @ Wrh split into mo-halves: half 0's
                                # tail (add + tanh of h chunks 0-3) overlaps
                                # half 1's matmuls, so the critical wait rides
                                # only half 1's semaphore backlog (PE sem incs
                                # serialize ~26ns/inst and lag a 64-MM burst
                                # by ~1us)
                                QCH = NCH // 4
                                for qu in range(4):
                                    pt = cp_pool.tile([P, QCH, B_LOC], FP32,
                                                      tag=f"pss{qu}", bufs=1)
                                    for m in range(QCH):
                                        mo = qu * QCH + m
                                        for ki in range(NCH):
                                            nc.tensor.matmul(
                                                pt[:, m, :],
                                                w_sb["rh"][:, ki, mo * P:(mo + 1) * P],
                                                uT[:, ki, :], start=(ki == 0),
                                                stop=(ki == NCH - 1))
                                    sl = slice(qu * QCH, (qu + 1) * QCH)
                                    if layer == 0:
                                        hb_h = hblk[:, j // SUB, sl, jj, :]
                                    else:
                                        hb_h = hblk[:, j // SUB, sl, :, jj]
                                    nc.vector.tensor_add(hb_h, pt, q_sb[:, sl, :])
                                    nc.scalar.activation(h_out[:, sl, :], hb_h,
                                                         AF.Tanh)
                                    nc.vector.tensor_add(s_sb[:, sl, :], pt,
                                                         s09[:, sl, :])

                            # block epilogue: bulk tanh + single-DMA store
                            if layer == 0:
                                ho = cb_pool.tile([P, 2, NCH, SUB, B_LOC], BF16, tag="hout")
                                nc.scalar.activation(ho, hblk, AF.Tanh)
                                nc.sync.dma_start(out=h1T[:, ds(bv, 2)], in_=ho)
                            else:
                                ho = cb_pool.tile([P, 2, NCH, B_LOC, SUB], FP32, tag="hout")
                                nc.scalar.activation(ho, hblk, AF.Tanh)
                                nc.sync.dma_start(out=h2T[:, ds(bv, 2)], in_=ho)

            # ---- Phase D: transpose layer-2 output back to [b, t, do]
            with tc.tile_pool(name="pd", bufs=3) as pd_pool, \
                 tc.tile_pool(name="pdp", bufs=2, space="PSUM") as pdp_pool:
                for b in range(B_LOC):
                    for c in range(NCH):          # do chunk
                        for tch in range(T // P):  # t chunk
                            ti = pd_pool.tile([P, P], FP32, tag="pd_in")
                            nsb = P // SUB
                            nc.sync.dma_start(
                                out=ti,
                                in_=h2T[:, tch * nsb:(tch + 1) * nsb, c, b, :])
                            pt = pdp_pool.tile([P, P], FP32, tag="pd_ps")
                            nc.tensor.transpose(pt, ti, ident)
                            to = pd_pool.tile([P, P], FP32, tag="pd_out")
                            nc.vector.tensor_copy(to, pt)
                            nc.sync.dma_start(
                                out=out[b, tch * P:(tch + 1) * P, c * P:(c + 1) * P],
                                in_=to)

    nc.finalize()
    return nc


def kernel(x_seq, W_xh, W_hh, W_rh, W_xr, W_hr):
    T = x_seq.shape[1]
    nc = build_nc(T=T)
    in_maps = []
    for i in range(N_CORES):
        in_maps.append({
            "x_seq": np.ascontiguousarray(x_seq[i * B_LOC:(i + 1) * B_LOC],
                                          dtype=np.float32),
            "W_xh": np.ascontiguousarray(W_xh, dtype=np.float32),
            "W_hh": np.ascontiguousarray(W_hh, dtype=np.float32),
            "W_rh": np.ascontiguousarray(W_rh, dtype=np.float32),
            "W_xr": np.ascontiguousarray(W_xr, dtype=np.float32),
            "W_hr": np.ascontiguousarray(W_hr, dtype=np.float32),
        })
    res = run_bass_kernel_spmd(nc, in_maps, core_ids=list(range(N_CORES)))
    outs = [r["out"] for r in res.results]
    return np.concatenate(outs, axis=0)
